# revision 48
# baseline (speedup 1.0000x reference)
"""Paged KV-cache decode attention with ALiBi (Baichuan-style), fused
QKV + attention + output projection, tensor-parallel over heads across
8 Trainium2 NeuronCores.

v2 design (bf16, minimal tensor-instruction count, DMA-roofline bound):
  - All matmuls in bf16: fp32 matmuls cost 2x (LDWEIGHTS+MATMUL) pairs
    on HW and 4x stream cycles; bf16 also halves HBM traffic.
  - Every matmul streams the BIG operand (weights/KV cache) as rhs with
    a tiny (1-4 col) lhsT weight load, maximizing work per instruction:
      * QKV:    lhsT = hT chunk [128,4],  rhs = W chunk [128,480]
      * scores: lhsT = qT col  [128,1],  rhs = K^T     [128,<=512]
      * attnV:  lhsT = attnT   [128,4],  rhs = V 4-pair blk [128,512]
      * o_proj: lhsT = aoT     [128,4],  rhs = W_o     [128,512]
  - ALiBi bias + causal mask folded multiplicatively: attn = exp(s) *
    ebias, ebias = exp(slope*(t-pos)) * (t<sl), host-precomputed (<=1,
    no overflow). Normalizer via one DVE tensor_tensor_reduce per pair.
  - Softmax normalization folded into the psum->sbuf extraction copies
    via activation(Copy, scale=recip[pair]).
  - Transposes (q/k, attn, attn-out) batched on the PE via identity
    matmuls, 31 total.
"""

import math
import os
import sys
from contextlib import ExitStack

import numpy as np
import ml_dtypes

sys.path.insert(0, "/opt/trn_rl_repo")

BF16 = ml_dtypes.bfloat16
FP8 = ml_dtypes.float8_e4m3

B = 4
E = 5120
H = 40
D = 128
BS = 16
NB = 512
MB = 128
S = MB * BS  # 2048
NCORES = 8
HPC = H // NCORES   # 5 heads per core
EPC = HPC * D       # 640
NP = HPC * B        # 20 (b,h) pairs per core; pair r = h*4 + b


def _alibi_slopes(num_heads):
    cp2 = 2 ** int(math.floor(math.log2(num_heads)))
    base = 2.0 ** (-(2.0 ** (-(math.log2(cp2) - 3))))
    slopes = base ** np.arange(1, cp2 + 1, dtype=np.float64)
    if cp2 != num_heads:
        extra_base = 2.0 ** (-(2.0 ** (-(math.log2(2 * cp2) - 3))))
        n_rem = min(cp2, num_heads - cp2)
        extra = extra_base ** np.arange(1, 1 + 2 * n_rem, 2, dtype=np.float64)
        slopes = np.concatenate([slopes, extra])
    return slopes.astype(np.float64)


_PROGRAM_CACHE = {}
LAST_RESULTS = None  # BassKernelResults of the most recent run (for test.py)


def _build_program(sl):
    """SPMD Bass program; per-sequence lengths sl baked statically."""
    import concourse.bacc as bacc
    import concourse.bass as bass
    import concourse.tile as tile
    from concourse import masks, mybir

    f32 = mybir.dt.float32
    bf16 = mybir.dt.bfloat16
    fp8 = mybir.dt.float8e4
    Exp = mybir.ActivationFunctionType.Exp
    Copy = mybir.ActivationFunctionType.Copy
    sm_scale = 1.0 / math.sqrt(D)

    pos = [s - 1 for s in sl]
    ncht = [(s + 127) // 128 for s in sl]     # 128-chunks per seq
    max_nch = max(ncht)

    nc = bacc.Bacc()

    hT = nc.declare_dram_parameter("hT", [128, 40 * B], bf16, isOutput=False)
    qkvw = nc.declare_dram_parameter("qkvw", [40, 128, 3 * EPC], bf16, isOutput=False)
    ow = nc.declare_dram_parameter("ow", [HPC, 128, E], bf16, isOutput=False)
    kt = nc.declare_dram_parameter("kt", [HPC, B, D, S], bf16, isOutput=False)
    vt = nc.declare_dram_parameter("vt", [HPC, 128, 16, B * D], bf16, isOutput=False)
    ebias = nc.declare_dram_parameter("ebias", [NP, S], bf16, isOutput=False)
    out = nc.declare_dram_parameter("out", [B, E], f32, isOutput=True)

    with tile.TileContext(nc) as tc, ExitStack() as ctx:
        consts = ctx.enter_context(tc.tile_pool(name="consts", bufs=1))
        wq = ctx.enter_context(tc.tile_pool(name="wq", bufs=3))
        wo = ctx.enter_context(tc.tile_pool(name="wo", bufs=2))
        kpool = ctx.enter_context(tc.tile_pool(name="kpool", bufs=1))
        vpool = ctx.enter_context(tc.tile_pool(name="vpool", bufs=3))

        # ---- constants / small inputs ----
        ident = consts.tile([32, 32], bf16)
        masks.make_identity(nc, ident[:])
        ones_col = consts.tile([128, 1], bf16)
        nc.vector.memset(ones_col[:], 1.0)
        hT_sb = consts.tile([128, 40 * B], bf16)
        nc.gpsimd.dma_start(out=hT_sb[:], in_=hT[:])

        qkT_sb = consts.tile([128, 2 * NP], bf16)   # q cols r, k cols 20+r
        attn_p = consts.tile([NP, S], bf16)         # exp(s)*ebias rows
        nc.vector.memset(attn_p[:], 0.0)
        attnT_sb = consts.tile([128, 16 * NP], bf16)
        rc4 = consts.tile([B, HPC], f32)            # 1/norm, [b, g]
        ao_nat = consts.tile([B, EPC], f32)         # attn-out rows (unnormalized)
        ao_bf = consts.tile([B, EPC], bf16)
        aoT_sb = consts.tile([128, NP], bf16)
        out_sb = consts.tile([B, E], f32)

        # K tiles all resident (exact per-seq size). Their DMAs are issued
        # AFTER the qkvw loop (below) so the qkvw stream gets the full DMA
        # bandwidth while the PE chews on it; K arrives during phase S.
        k_tiles = {}
        for g in range(HPC):
            for b in range(B):
                t = kpool.tile([128, sl[b]], bf16, tag=f"K{g}_{b}",
                               name=f"K_{g}_{b}")
                k_tiles[(g, b)] = t
        ngv = max_nch
        v_tiles = [vpool.tile([128, ngv * B * D], bf16, tag="V", name=f"V{g}")
                   for g in range(HPC)]

        # ---- Phase Q: fused QKV projection. The 4 psum quarters live at
        # PE column-groups {0,32,64,96} of ONE bank (tile_position col
        # tiling) so they stream concurrently through the array. ----
        with tc.tile_pool(name="psq", bufs=1, space="PSUM") as psq:
            qkv_ps = psq.tile([128, 480], f32, tag="ps")
            for kc in range(40):
                wt = wq.tile([128, 3 * EPC], bf16, tag="w")
                nc.sync.dma_start(out=wt[:], in_=qkvw[kc])
                for j in range(4):
                    nc.tensor.matmul(
                        qkv_ps[32 * j: 32 * j + 4, :],
                        lhsT=hT_sb[:, kc * 4:(kc + 1) * 4],
                        rhs=wt[:, j * 480:(j + 1) * 480],
                        start=(kc == 0),
                        stop=(kc == 39),
                        tile_position=(0, 32 * j),
                    )
            # K then first-3 V loads on the SAME sync ring as qkvw: HWDGE
            # rings are FIFO, so these transfers start exactly when the
            # qkvw stream finishes - qkvw gets full bandwidth, K arrives
            # just in time for phase S, V during it.
            for g in range(HPC):
                for b in range(B):
                    nc.sync.dma_start(
                        out=k_tiles[(g, b)][:], in_=kt[g, b, :, : sl[b]]
                    )
            for g in range(3):
                nc.sync.dma_start(
                    out=v_tiles[g][:], in_=vt[g, :, :ngv, :]
                )
            qkv_nat = consts.tile([B, 3 * EPC], bf16)
            for j in range(4):
                nc.scalar.copy(qkv_nat[:, j * 480:(j + 1) * 480],
                               qkv_ps[32 * j: 32 * j + 4, :])

            # transpose q,k -> [128(d), 20] with col r = h*4+b
            qkT_ps = psq.tile([128, 2 * NP], bf16, tag="tp")
            for w in range(2):
                for h in range(HPC):
                    nc.tensor.matmul(
                        qkT_ps[:, w * NP + h * 4: w * NP + (h + 1) * 4],
                        lhsT=qkv_nat[:, w * EPC + h * D: w * EPC + (h + 1) * D],
                        rhs=ident[:B, :B],
                        is_transpose=True,
                    )
            nc.scalar.copy(qkT_sb[:], qkT_ps[:])
        v_sb = qkv_nat[:, 2 * EPC: 3 * EPC]  # natural v rows [4, 640]

        # ---- Phase S: scores + exp + ebias-mult (per pair) ----
        # Engine ops may only address partition base 0, so each pair works
        # in its own [1, S] tiles; rows are assembled into attn_p / rc4 by
        # tiny SBUF->SBUF DMAs (which may target any partition).
        rowpool = ctx.enter_context(tc.tile_pool(name="rowpool", bufs=2))
        with tc.tile_pool(name="pss", bufs=2, space="PSUM") as pss:
            for g in range(HPC):
                for b in range(B):
                    r = g * 4 + b
                    Ktile = k_tiles[(g, b)]
                    # scatter new-token k column (same partitions)
                    nc.vector.tensor_copy(
                        Ktile[:, pos[b]: pos[b] + 1], qkT_sb[:, NP + r: NP + r + 1]
                    )
                    s_ps = pss.tile([1, S], f32, tag="s")
                    nq = (sl[b] + 511) // 512
                    for qq in range(nq):
                        ncols = min(512, sl[b] - qq * 512)
                        nc.tensor.matmul(
                            s_ps[:, qq * 512: qq * 512 + ncols],
                            lhsT=qkT_sb[:, r: r + 1],
                            rhs=Ktile[:, qq * 512: qq * 512 + ncols],
                            start=True,
                            stop=True,
                        )
                    eb_t = rowpool.tile([1, S], bf16, tag="eb")
                    nc.gpsimd.dma_start(
                        out=eb_t[:, : sl[b]], in_=ebias[r: r + 1, : sl[b]]
                    )
                    ae_t = rowpool.tile([1, S], bf16, tag="ae")
                    nc.scalar.activation(
                        ae_t[:, : sl[b]], s_ps[:, : sl[b]], func=Exp,
                        scale=sm_scale,
                    )
                    ap_t = rowpool.tile([1, S], bf16, tag="ap")
                    nc.vector.tensor_mul(
                        ap_t[:, : sl[b]], ae_t[:, : sl[b]], eb_t[:, : sl[b]]
                    )
                    # assemble rows at their pair offsets via DMA (scalar
                    # ring: small + latency-sensitive, won't queue behind
                    # the bulk streams on the sync ring)
                    nc.scalar.dma_start(
                        out=attn_p[r: r + 1, : sl[b]], in_=ap_t[:, : sl[b]]
                    )

        # ---- Phase T: transpose attn rows -> attnT [128(t%128), (c, r)],
        # and softmax normalizers via ones-column matmuls over attnT ----
        recip_sb = consts.tile([NP, 1], f32)
        with tc.tile_pool(name="pst", bufs=1, space="PSUM") as pst:
            attnT_ps = pst.tile([128, 16 * NP], bf16, tag="tp")
            for c in range(max_nch):
                nc.tensor.matmul(
                    attnT_ps[:, c * NP:(c + 1) * NP],
                    lhsT=attn_p[:, c * 128:(c + 1) * 128],
                    rhs=ident[:NP, :NP],
                    is_transpose=True,
                )
            nc.scalar.copy(
                attnT_sb[:, : max_nch * NP], attnT_ps[:, : max_nch * NP]
            )
            norm_ps = pst.tile([NP, 1], f32, tag="nrm")
            for c in range(max_nch):
                nc.tensor.matmul(
                    norm_ps[:],
                    lhsT=attnT_sb[:, c * NP:(c + 1) * NP],
                    rhs=ones_col[:],
                    start=(c == 0),
                    stop=(c == max_nch - 1),
                )
            nc.vector.reciprocal(recip_sb[:], norm_ps[:])
        for g in range(HPC):
            for b in range(B):
                r = g * 4 + b
                nc.gpsimd.dma_start(
                    out=rc4[b: b + 1, g: g + 1], in_=recip_sb[r: r + 1, :]
                )

        # ---- Phase A: attn @ V per head-group (4 pairs per matmul) ----
        ng = max_nch
        with tc.tile_pool(name="psa", bufs=2, space="PSUM") as psa:
            for g in range(HPC):
                Vg = v_tiles[g]
                if g >= 3:
                    nc.sync.dma_start(out=Vg[:], in_=vt[g, :, :ng, :])
                # scatter new-token v rows (cross-partition -> DMA)
                for b in range(B):
                    p = pos[b]
                    nc.gpsimd.dma_start(
                        out=Vg[p % 128: p % 128 + 1,
                               (p // 128) * B * D + b * D: (p // 128) * B * D + (b + 1) * D],
                        in_=v_sb[b: b + 1, g * D:(g + 1) * D],
                    )
                ao_ps = psa.tile([B, B * D], f32, tag="ao")
                for c in range(ng):
                    nc.tensor.matmul(
                        ao_ps[:],
                        lhsT=attnT_sb[:, c * NP + g * 4: c * NP + (g + 1) * 4],
                        rhs=Vg[:, c * B * D:(c + 1) * B * D],
                        start=(c == 0),
                        stop=(c == ng - 1),
                    )
                # evict psum -> sbuf, then diag-extract rows via DMA
                ao_full = rowpool.tile([B, B * D], f32, tag="aof")
                nc.scalar.copy(ao_full[:], ao_ps[:])
                for b in range(B):
                    nc.gpsimd.dma_start(
                        out=ao_nat[b: b + 1, g * D:(g + 1) * D],
                        in_=ao_full[b: b + 1, b * D:(b + 1) * D],
                    )
            # normalize (per-partition scalar = 1/norm) + cast to bf16
            for g in range(HPC):
                nc.vector.tensor_scalar_mul(
                    ao_bf[:, g * D:(g + 1) * D],
                    ao_nat[:, g * D:(g + 1) * D],
                    rc4[:, g: g + 1],
                )

            # transpose attn-out -> aoT [128(d), 20] col h*4+b
            aoT_ps = psa.tile([128, NP], bf16, tag="aot")
            for h in range(HPC):
                nc.tensor.matmul(
                    aoT_ps[:, h * 4:(h + 1) * 4],
                    lhsT=ao_bf[:, h * D:(h + 1) * D],
                    rhs=ident[:B, :B],
                    is_transpose=True,
                )
            nc.scalar.copy(aoT_sb[:], aoT_ps[:])

        # ---- Phase O: output projection. 10 col-blocks packed 4-wide via
        # tile_position across 3 psum banks - no waves, ow streamed once. ----
        with tc.tile_pool(name="pso", bufs=3, space="PSUM") as pso:
            o_ps = [pso.tile([128, 512], f32, tag="o", name=f"ob{i}")
                    for i in range(3)]
            for h in range(HPC):
                owt = wo.tile([128, E], bf16, tag="ow", name=f"ow_{h}")
                nc.sync.dma_start(out=owt[:], in_=ow[h])
                for j in range(10):
                    cg = 32 * (j % 4)
                    nc.tensor.matmul(
                        o_ps[j // 4][cg: cg + 4, :],
                        lhsT=aoT_sb[:, h * 4:(h + 1) * 4],
                        rhs=owt[:, j * 512:(j + 1) * 512],
                        start=(h == 0),
                        stop=(h == HPC - 1),
                        tile_position=(0, cg),
                    )
            for j in range(10):
                cg = 32 * (j % 4)
                nc.scalar.copy(out_sb[:, j * 512:(j + 1) * 512],
                               o_ps[j // 4][cg: cg + 4, :])
                nc.gpsimd.dma_start(
                    out=out[:, j * 512:(j + 1) * 512],
                    in_=out_sb[:, j * 512:(j + 1) * 512],
                )

    nc.compile()
    return nc


def _prepare_core_inputs(core, hidden_bf, qkv_w, o_w, k_cache, v_cache, bt, sl):
    hs = slice(core * HPC, (core + 1) * HPC)
    es = slice(core * EPC, (core + 1) * EPC)

    # qkvw: [40, 128, 1920] bf16, cols [q | k | v]; sm_scale is applied
    # at the exp on-device rather than folded into wq.
    qkvw = np.concatenate(
        [qkv_w[0][:, es], qkv_w[1][:, es], qkv_w[2][:, es]], axis=1
    ).astype(BF16)
    qkvw = np.ascontiguousarray(qkvw.reshape(40, 128, 3 * EPC))

    ow_s = np.ascontiguousarray(
        o_w[es, :].astype(BF16).reshape(HPC, 128, E)
    )

    kg = k_cache[:, hs]  # [NB, HPC, BS, D]
    vg = v_cache[:, hs]
    kt = np.empty((HPC, B, D, S), BF16)
    vt = np.empty((HPC, 128, 16, B, D), BF16)
    for b in range(B):
        kk = kg[bt[b]].transpose(1, 0, 2, 3).reshape(HPC, S, D)  # [h, t, d]
        kt[:, b] = kk.transpose(0, 2, 1).astype(BF16)
        vv = vg[bt[b]].transpose(1, 0, 2, 3).reshape(HPC, 16, 128, D)
        vt[:, :, :, b, :] = vv.transpose(0, 2, 1, 3).astype(BF16)

    slopes = _alibi_slopes(H)[core * HPC:(core + 1) * HPC]
    t_idx = np.arange(S, dtype=np.float64)
    eb = np.zeros((HPC, B, S), np.float32)
    for h in range(HPC):
        for b in range(B):
            ex = np.minimum(slopes[h] * (t_idx - np.float64(sl[b] - 1)), 0.0)
            v = np.exp(ex).astype(np.float32)
            v[t_idx >= sl[b]] = 0.0
            eb[h, b] = v
    ebias = np.ascontiguousarray(eb.reshape(NP, S).astype(BF16))

    return dict(
        hT=hidden_bf,
        qkvw=qkvw,
        ow=ow_s,
        kt=np.ascontiguousarray(kt),
        vt=np.ascontiguousarray(vt.reshape(HPC, 128, 16, B * D)),
        ebias=ebias,
    )


def kernel(**inputs):
    global LAST_RESULTS
    hidden = np.asarray(inputs["hidden_states"], np.float32)
    qkv_w = np.asarray(inputs["qkv_weight"], np.float32)
    o_w = np.asarray(inputs["o_proj_weight"], np.float32)
    k_cache = np.asarray(inputs["k_cache"], np.float32)
    v_cache = np.asarray(inputs["v_cache"], np.float32)
    bt = np.asarray(inputs["block_tables"]).astype(np.int64)
    sl_arr = np.asarray(inputs["sequence_lengths"]).astype(np.int64)
    sl = tuple(int(x) for x in sl_arr)

    # hT[p, kc*4+b] = hidden[b, kc*128+p]
    hidden_bf = np.ascontiguousarray(
        hidden.T.reshape(40, 128, B).transpose(1, 0, 2).reshape(128, 40 * B)
    ).astype(BF16)

    in_maps = [
        _prepare_core_inputs(c, hidden_bf, qkv_w, o_w, k_cache, v_cache, bt, sl)
        for c in range(NCORES)
    ]

    if sl not in _PROGRAM_CACHE:
        _PROGRAM_CACHE[sl] = _build_program(sl)
    nc = _PROGRAM_CACHE[sl]

    from concourse.bass_utils import run_bass_kernel_spmd

    res = run_bass_kernel_spmd(
        nc,
        in_maps,
        core_ids=list(range(NCORES)),
        trace=bool(os.environ.get("BASS_TRACE")),
    )
    LAST_RESULTS = res

    acc = np.zeros((B, E), np.float64)
    for c in range(NCORES):
        acc += np.asarray(res.results[c]["out"]).astype(np.float64)
    return acc.astype(np.float32)
